# revision 33
# baseline (speedup 1.0000x reference)
"""Accurate SDF (garment-to-body signed distance) on 8 Trainium2 cores — v2.

Faces sharded 8 ways (1722/core, padded to 14*128); every core scores all
B*G garment points against its faces and returns per-PSUM-partition running
minima [B, 128, G] (no on-device argmin). Host takes the top-M partitions
per point by device score, exactly re-ranks their 14 faces each in fp64,
and finishes (region code, normals, sign) with the reference formulas.

Device math per (face f, point g), with faces on partitions and g on the
free dim (moving rows P5 = [px, py, pz, 1, |p|^2]):
  edge e (seg anchor v_e, unit dir u_e, length L_e):
    U_e = u_e.(p - v_e)                (fp32 matmul)
    T_e = clamp(U_e, 0, L_e)           (relu on Act + min on DVE/Pool)
    w_e = T_e*(2U_e - T_e)             so d2_e = |p - v_e|^2 - w_e
  A    = |p - a|^2                     (fp32 matmul, |p|^2 row)
  A_b  = A + D',  D' = -2 L_ab U_ab + L_ab^2   (Act scale/bias from U_ab)
  face: h = n^.(p - a)  (fp32 matmul), score h^2, masked by the sign of
    vb, vc, va = den - vb - vc (row-normalized fp32r matmuls) via a
    BIG*relu(-min(...)) penalty.
  sc = min(A - max(w_ab, w_ca), A_b - w_bc, h^2 + penalty)
  best[partition] = min over ft tiles  ->  DMA out per (b, gchunk).
"""

import numpy as np

B, G, V, F = 2, 1024, 6890, 13776
NCORES = 8
FC = F // NCORES            # 1722 faces per core
FTILES = 14                 # ceil(1722/128)
FPAD = FTILES * 128         # 1792
GCHUNK = 512
NMM5 = 5                    # fp32 matmuls: U_ab, U_ca, U_bc, A, h
NMM3 = 3                    # fp32r matmuls: vb, vc, va
NCST = 5                    # ptr consts: L_ab, L_ca, L_bc, -2L_ab, L_ab^2
W5COLS = B * FTILES * 2 * 128      # 2 col-blocks, 3+2 sets at part 0/32/64
W3COLS = B * FTILES * 128          # 1 col-block, 3 sets at part 0/32/64
NPART = 69                         # weight/moving tiles span partitions 0..68
CSTCOLS = B * FTILES * NCST
BIG = np.float32(1e6)
INF = np.float32(3e38)
TOPM = 32                   # host: partitions re-ranked exactly per point

_CACHE = {}


def _build_bass():
    import concourse.bass as bass
    import concourse.bacc as bacc
    import concourse.mybir as mybir
    from concourse.tile import TileContext

    dt = mybir.dt.float32
    dtr = mybir.dt.float32r
    Alu = mybir.AluOpType
    Act = mybir.ActivationFunctionType

    nc = bacc.Bacc()

    w5h_d = nc.declare_dram_parameter("w5h", [NPART, W5COLS], dtr, isOutput=False)
    w5l_d = nc.declare_dram_parameter("w5l", [NPART, W5COLS], dtr, isOutput=False)
    w3_d = nc.declare_dram_parameter("w3r", [NPART, W3COLS], dtr, isOutput=False)
    p5h_d = nc.declare_dram_parameter("p5h", [NPART, B * G], dtr, isOutput=False)
    p5l_d = nc.declare_dram_parameter("p5l", [NPART, B * G], dtr, isOutput=False)
    cst_d = nc.declare_dram_parameter("cst", [128, CSTCOLS], dt, isOutput=False)
    oval_d = nc.declare_dram_parameter("out_val", [B, FTILES, 128, G], dt,
                                   isOutput=True)

    Vv = nc.vector
    Gg = nc.gpsimd
    Ss = nc.scalar
    Tt = nc.tensor
    Sy = nc.sync

    with TileContext(nc) as tc:
        with (
            tc.tile_pool(name="cpool", bufs=1) as cpool,
            tc.tile_pool(name="work", bufs=1) as work,
            tc.tile_pool(name="mm", bufs=2, space="PSUM") as mm,
        ):
            w5h_s = cpool.tile([NPART, W5COLS], dtr, name="w5h_s")
            w5l_s = cpool.tile([NPART, W5COLS], dtr, name="w5l_s")
            w3_s = cpool.tile([NPART, W3COLS], dtr, name="w3_s")
            p5h_s = cpool.tile([NPART, B * G], dtr, name="p5h_s")
            p5l_s = cpool.tile([NPART, B * G], dtr, name="p5l_s")
            cst_s = cpool.tile([128, CSTCOLS], dt, name="cst_s")
            Sy.dma_start(w5h_s[:], w5h_d[:])
            Sy.dma_start(w5l_s[:], w5l_d[:])
            Sy.dma_start(w3_s[:], w3_d[:])
            Sy.dma_start(p5h_s[:], p5h_d[:])
            Sy.dma_start(p5l_s[:], p5l_d[:])
            Sy.dma_start(cst_s[:], cst_d[:])

            def W5(b, ft, m, lo=False):
                blk, off = divmod(m, 3)
                c = ((b * FTILES + ft) * 2 + blk) * 128
                ws = w5l_s if lo else w5h_s
                return ws[off * 32:off * 32 + NMM5, c:c + 128]

            def W3(b, ft, m):
                c = (b * FTILES + ft) * 128
                return w3_s[m * 32:m * 32 + NMM5, c:c + 128]

            def CST(b, ft, j):
                c = (b * FTILES + ft) * NCST + j
                return cst_s[:, c:c + 1]

            GW = 2 * GCHUNK                     # fused width: both gchunks of b
            sh = [128, GW]

            def emit(b, ft):
                """One face-tile iteration over all G points of batch b.

                Matmuls run as 2x512-wide halves into [128,1024] psum tiles
                (2 banks each); two psum names x bufs=2 = 8 banks, rotated so
                every wait lands on an early Act drain. All elementwise ops
                run fused at width 1024."""
                g0 = b * G

                def MM(name, m, compensated=True):
                    t = mm.tile(sh, dt, name=name)
                    for h in range(2):
                        dst = t[:, h * GCHUNK:(h + 1) * GCHUNK]
                        if compensated:
                            blk, off = divmod(m, 3)
                            ph = p5h_s[off * 32:off * 32 + NMM5,
                                       g0 + h * GCHUNK:g0 + (h + 1) * GCHUNK]
                            pl = p5l_s[off * 32:off * 32 + NMM5,
                                       g0 + h * GCHUNK:g0 + (h + 1) * GCHUNK]
                            Tt.matmul(dst, W5(b, ft, m), ph,
                                      start=True, stop=False)
                            Tt.matmul(dst, W5(b, ft, m), pl,
                                      start=False, stop=False)
                            Tt.matmul(dst, W5(b, ft, m, lo=True), ph,
                                      start=False, stop=True)
                        else:
                            ph = p5h_s[m * 32:m * 32 + NMM5,
                                       g0 + h * GCHUNK:g0 + (h + 1) * GCHUNK]
                            Tt.matmul(dst, W3(b, ft, m), ph,
                                      start=True, stop=True)
                    return t

                def wt(nm, bufs=1):
                    return work.tile(sh, dt, name=nm, bufs=bufs)

                u_ab = MM("p1", 0)
                u_ca = MM("p2", 1)
                u_bc = MM("p1", 2)
                vcm = MM("p2", 1, compensated=False)
                am = MM("p1", 3)
                hm = MM("p2", 4)
                abm = MM("p1", 5)
                vbm = MM("p2", 0, compensated=False)
                vam = MM("p1", 2, compensated=False)
                # Act: drain psum fast (order matches psum rotation)
                r1ab = wt("r1ab", bufs=2); r1ca = wt("r1ca", bufs=2)
                r1bc = wt("r1bc")
                sf = wt("sf"); ab_s = wt("ab_s")
                a_s = wt("a_s")
                vb_s = wt("vb_s"); vc_s = wt("vc_s"); va_s = wt("va_s")
                Ss.activation(r1ab[:], u_ab[:], Act.Relu)
                Ss.activation(r1ca[:], u_ca[:], Act.Relu)
                Ss.activation(r1bc[:], u_bc[:], Act.Relu)
                Ss.activation(vc_s[:], vcm[:], Act.Identity)
                Ss.activation(a_s[:], am[:], Act.Identity)
                Ss.activation(sf[:], hm[:], Act.Square)
                Ss.activation(ab_s[:], abm[:], Act.Identity)
                Ss.activation(vb_s[:], vbm[:], Act.Identity)
                Ss.activation(va_s[:], vam[:], Act.Identity)
                # clamp T = min(relu(U), L)
                t_ab = wt("t_ab"); t_ca = wt("t_ca"); t_bc = wt("t_bc")
                Vv.tensor_scalar(t_ab[:], r1ab[:], CST(b, ft, 0), None, Alu.min)
                Vv.tensor_scalar(t_ca[:], r1ca[:], CST(b, ft, 1), None, Alu.min)
                Vv.tensor_scalar(t_bc[:], r1bc[:], CST(b, ft, 2), None, Alu.min)
                # z = 2*relu(U) - T  (== 2U - T wherever T != 0)
                z_ab = wt("z_ab"); z_ca = wt("z_ca"); z_bc = wt("z_bc")
                Vv.scalar_tensor_tensor(z_ab[:], r1ab[:], 2.0, t_ab[:],
                                        Alu.mult, Alu.subtract)
                Vv.scalar_tensor_tensor(z_ca[:], r1ca[:], 2.0, t_ca[:],
                                        Alu.mult, Alu.subtract)
                Vv.scalar_tensor_tensor(z_bc[:], r1bc[:], 2.0, t_bc[:],
                                        Alu.mult, Alu.subtract)
                w_ab = wt("w_ab"); w_ca = wt("w_ca")
                w_bc = wt("w_bc")
                Gg.tensor_tensor(w_ab[:], t_ab[:], z_ab[:], Alu.mult)
                Gg.tensor_tensor(w_ca[:], t_ca[:], z_ca[:], Alu.mult)
                Gg.tensor_tensor(w_bc[:], t_bc[:], z_bc[:], Alu.mult)
                # face mask: penalty = BIG*relu(-min(vb,vc,va))
                mn1 = wt("mn1"); mn2 = wt("mn2"); rneg = wt("rneg")
                sfm = wt("sfm")
                Vv.tensor_tensor(mn1[:], vb_s[:], vc_s[:], Alu.min)
                Vv.tensor_tensor(mn2[:], mn1[:], va_s[:], Alu.min)
                Ss.activation(rneg[:], mn2[:], Act.Relu, scale=-float(BIG))
                # combine
                mx = wt("mx"); e1 = wt("e1"); e2 = wt("e2")
                em = wt("em"); sc = wt("sc", bufs=2)
                Vv.tensor_tensor(sfm[:], rneg[:], sf[:], Alu.add)
                Vv.tensor_tensor(mx[:], w_ab[:], w_ca[:], Alu.max)
                Gg.tensor_tensor(e1[:], a_s[:], mx[:], Alu.subtract)
                Gg.tensor_tensor(e2[:], ab_s[:], w_bc[:], Alu.subtract)
                Vv.tensor_tensor(em[:], e1[:], e2[:], Alu.min)
                Vv.tensor_tensor(sc[:], em[:], sfm[:], Alu.min)
                Sy.dma_start(oval_d[b, ft], sc[:])

            for b in range(B):
                for ft in range(FTILES):
                    emit(b, ft)
    nc.finalize()
    return nc


def _get_nc():
    if "nc" not in _CACHE:
        _CACHE["nc"] = _build_bass()
    return _CACHE["nc"]


def _round_fp32r(x):
    """Round fp32 -> fp32r container (11-bit mantissa, RNE)."""
    u = np.ascontiguousarray(x, np.float32).view(np.uint32)
    base = u & np.uint32(0xFFFFF000)
    low = u & np.uint32(0x00000FFF)
    half = np.uint32(0x800)
    lsb = (base >> np.uint32(12)) & np.uint32(1)
    up = (low > half) | ((low == half) & (lsb == 1))
    return np.where(up, base + np.uint32(0x1000), base).view(np.float32)


def _core_inputs(batch_garment_verts, batch_body_verts, body_faces):
    f32 = np.float32
    gv = batch_garment_verts.astype(f32)
    p5 = np.concatenate(
        [gv.transpose(0, 2, 1),                       # [B,3,G]
         np.ones((B, 1, G), f32),
         np.sum(gv * gv, -1, dtype=f32)[:, None, :]], 1
    ).reshape(B * NMM5, G)
    p5 = np.ascontiguousarray(
        p5.reshape(B, NMM5, G).transpose(1, 0, 2)).reshape(NMM5, B * G)
    p5x = np.zeros((69, B * G), f32)            # replicate at part 0/32/64
    for o in range(3):
        p5x[o * 32:o * 32 + NMM5] = p5
    p5 = p5x
    p5h = _round_fp32r(p5)
    p5l = _round_fp32r(p5 - p5h)

    in_maps = []
    for c in range(NCORES):
        sl = slice(c * FC, (c + 1) * FC)
        w5 = np.zeros((69, W5COLS), f32)
        w3 = np.zeros((69, W3COLS), f32)
        cst = np.zeros((128, CSTCOLS), f32)
        for b in range(B):
            fv = batch_body_verts[b].astype(f32)[body_faces[sl]]  # [FC,3,3]
            a, bb, cc = fv[:, 0], fv[:, 1], fv[:, 2]
            n = np.cross((bb - a).astype(np.float64),
                         (cc - a).astype(np.float64))
            nn = np.linalg.norm(n, axis=1)
            ab, ac = (bb - a).astype(f32), (cc - a).astype(f32)
            naa = np.sum(ab * ab, -1, dtype=f32)
            nab = np.sum(ab * ac, -1, dtype=f32)
            ncc = np.sum(ac * ac, -1, dtype=f32)
            den = (naa * ncc - nab * nab).astype(f32)
            degen = (den < f32(1e-4)) | (nn < 1e-10)
            nh = np.where(degen[:, None], 0,
                          n / np.maximum(nn, 1e-30)[:, None]).astype(f32)

            def pads(rows):  # [FC,5] -> [5, FPAD] padded
                out = np.zeros((NMM5, FPAD), f32)
                out[:, :FC] = rows.T
                return out

            Ls = {}
            Ws = {}
            for nm, (ea, eb) in (("ab", (a, bb)), ("ca", (a, cc)),
                                 ("bc", (bb, cc))):
                ed = (eb - ea).astype(f32)
                L = np.linalg.norm(ed.astype(np.float64), axis=1).astype(f32)
                u = np.where(L[:, None] > 0,
                             ed / np.maximum(L, f32(1e-30))[:, None], 0)
                Ws[nm] = pads(np.concatenate(
                    [u, (-np.sum(u * ea, -1, dtype=f32))[:, None],
                     np.zeros((FC, 1), f32)], 1))
                Lp = np.zeros(FPAD, f32)
                Lp[:FC] = L
                Ls[nm] = Lp
            wa_rows = np.concatenate(
                [-2 * a, np.sum(a * a, -1, dtype=f32)[:, None],
                 np.ones((FC, 1), f32)], 1)
            WA = pads(wa_rows)
            WA[3, FC:] = f32(1e30)                       # pad faces: huge A
            wab_rows = np.concatenate(
                [-2 * bb, np.sum(bb * bb, -1, dtype=f32)[:, None],
                 np.ones((FC, 1), f32)], 1)
            WAb = pads(wab_rows)
            WAb[3, FC:] = f32(1e30)
            Wh = pads(np.concatenate(
                [nh, (-np.sum(nh * a, -1, dtype=f32))[:, None],
                 np.zeros((FC, 1), f32)], 1))
            wvb = (ncc[:, None] * ab - nab[:, None] * ac).astype(f32)
            wvc = (naa[:, None] * ac - nab[:, None] * ab).astype(f32)
            cvb = (-np.sum(wvb * a, -1, dtype=f32))
            cvc = (-np.sum(wvc * a, -1, dtype=f32))
            wva = (-(wvb + wvc)).astype(f32)
            cva = (den - cvb - cvc).astype(f32)
            W3L = []
            for w_, c_ in ((wvb, cvb), (wvc, cvc), (wva, cva)):
                s = np.maximum(np.linalg.norm(w_, axis=1), f32(1e-30)).astype(f32)
                W3L.append(pads(np.concatenate(
                    [w_ / s[:, None], (c_ / s)[:, None],
                     np.zeros((FC, 1), f32)], 1)))
            Wvb, Wvc, Wva = W3L
            Wva[:, :FC][:, degen] = 0.0
            Wva[3, :FC][degen] = -1.0
            Wva[:, FC:] = 0.0
            Wva[3, FC:] = -1.0                           # pad faces: outside

            mm5 = [Ws["ab"], Ws["ca"], Ws["bc"], WA, Wh, WAb]
            mm3 = [Wvb, Wvc, Wva]
            for ft in range(FTILES):
                fsl = slice(ft * 128, (ft + 1) * 128)
                for m in range(6):
                    blk, off = divmod(m, 3)
                    c0 = ((b * FTILES + ft) * 2 + blk) * 128
                    w5[off * 32:off * 32 + NMM5, c0:c0 + 128] = mm5[m][:, fsl]
                for m in range(NMM3):
                    c0 = (b * FTILES + ft) * 128
                    w3[m * 32:m * 32 + NMM5, c0:c0 + 128] = mm3[m][:, fsl]
                c0 = (b * FTILES + ft) * NCST
                cst[:, c0 + 0] = Ls["ab"][fsl]
                cst[:, c0 + 1] = Ls["ca"][fsl]
                cst[:, c0 + 2] = Ls["bc"][fsl]
                cst[:, c0 + 3] = -2 * Ls["ab"][fsl]
                cst[:, c0 + 4] = Ls["ab"][fsl] ** 2
        w5h = _round_fp32r(w5)
        w5l = _round_fp32r(w5 - w5h)
        in_maps.append({"w5h": w5h, "w5l": w5l, "w3r": _round_fp32r(w3),
                        "p5h": p5h, "p5l": p5l, "cst": cst})
    return in_maps


def _d2_exact64_cand(p, bverts, faces, cand):
    """Exact fp64 point-triangle dist^2 for candidate faces. cand [G,C]."""
    fv = bverts[faces[cand]].astype(np.float64)      # [G,C,3,3]
    a, b, c = fv[:, :, 0], fv[:, :, 1], fv[:, :, 2]
    q = p.astype(np.float64)[:, None, :]
    best = np.full(cand.shape, np.inf)
    for ea, eb in ((a, b), (b, c), (c, a)):
        ed = eb - ea
        L2 = np.sum(ed * ed, -1)
        pe = q - ea
        t = np.clip(np.sum(pe * ed, -1) / np.maximum(L2, 1e-300), 0, 1)
        d = pe - t[..., None] * ed
        best = np.minimum(best, np.sum(d * d, -1))
    ab, ac = b - a, c - a
    n = np.cross(ab, ac)
    naa = np.sum(ab * ab, -1); nab = np.sum(ab * ac, -1)
    ncc = np.sum(ac * ac, -1)
    den = naa * ncc - nab * nab
    pa = q - a
    d1 = np.sum(pa * ab, -1); d2_ = np.sum(pa * ac, -1)
    vb = ncc * d1 - nab * d2_; vc = naa * d2_ - nab * d1
    va = den - vb - vc
    inside = (vb >= 0) & (vc >= 0) & (va >= 0) & (den > 1e-300)
    hn = np.sum(pa * n, -1)
    h2 = hn * hn / np.maximum(den, 1e-300)
    return np.where(inside, np.minimum(best, h2), best)


def _host_finish(g_verts, b_verts, faces, tri):
    """Exact reference finish for the winning face of each garment point."""
    f32 = np.float32
    EPS = f32(1e-10)

    def safe(x):
        return np.where(np.abs(x) < 1e-12, f32(1e-12), x).astype(f32)

    fverts = b_verts[faces]
    a_, b_, c_ = fverts[:, 0], fverts[:, 1], fverts[:, 2]
    fn_raw = np.cross(b_ - a_, c_ - a_).astype(f32)
    vn = np.zeros_like(b_verts)
    for k in range(3):
        np.add.at(vn, faces[:, k], fn_raw)
    vn = vn / (np.linalg.norm(vn, axis=-1, keepdims=True).astype(f32) + EPS)
    fn = fn_raw / (np.linalg.norm(fn_raw, axis=-1, keepdims=True).astype(f32) + EPS)

    a = a_[tri]; bb = b_[tri]; cc = c_[tri]
    q = g_verts
    ab = bb - a; ac = cc - a
    ap = q - a
    d1 = np.sum(ab * ap, -1); d2 = np.sum(ac * ap, -1)
    bp = q - bb
    d3 = np.sum(ab * bp, -1); d4 = np.sum(ac * bp, -1)
    cp = q - cc
    d5 = np.sum(ab * cp, -1); d6 = np.sum(ac * cp, -1)
    vc = d1 * d4 - d3 * d2
    vb = d5 * d2 - d1 * d6
    va = d3 * d6 - d5 * d4
    denom = safe(va + vb + vc)
    v, w = (vb / denom).astype(f32), (vc / denom).astype(f32)
    part = np.zeros(v.shape, np.int32)
    t_bc = ((d4 - d3) / safe((d4 - d3) + (d5 - d6))).astype(f32)
    m = (va <= 0) & (d4 - d3 >= 0) & (d5 - d6 >= 0)
    v = np.where(m, 1.0 - t_bc, v).astype(f32)
    w = np.where(m, t_bc, w).astype(f32)
    part = np.where(m, 2, part)
    t_ac = (d2 / safe(d2 - d6)).astype(f32)
    m = (vb <= 0) & (d2 >= 0) & (d6 <= 0)
    v = np.where(m, 0.0, v).astype(f32)
    w = np.where(m, t_ac, w).astype(f32)
    part = np.where(m, 3, part)
    m = (d6 >= 0) & (d5 <= d6)
    v = np.where(m, 0.0, v).astype(f32)
    w = np.where(m, 1.0, w).astype(f32)
    part = np.where(m, 6, part)
    t_ab = (d1 / safe(d1 - d3)).astype(f32)
    m = (vc <= 0) & (d1 >= 0) & (d3 <= 0)
    v = np.where(m, t_ab, v).astype(f32)
    w = np.where(m, 0.0, w).astype(f32)
    part = np.where(m, 1, part)
    m = (d3 >= 0) & (d4 <= d3)
    v = np.where(m, 1.0, v).astype(f32)
    w = np.where(m, 0.0, w).astype(f32)
    part = np.where(m, 5, part)
    m = (d1 <= 0) & (d2 <= 0)
    v = np.where(m, 0.0, v).astype(f32)
    w = np.where(m, 0.0, w).astype(f32)
    part = np.where(m, 4, part)
    npt = a + v[:, None] * ab + w[:, None] * ac

    fidx = faces[tri]
    gar = np.arange(len(tri))
    take = lambda col: vn[fidx[gar, col]]
    n_face = fn[tri]
    n_vert = take(np.clip(part - 4, 0, 2))
    n_edge = take(np.clip(part - 1, 0, 2)) + take(np.mod(part, 3))
    n = np.where((part == 0)[:, None], n_face,
                 np.where((part > 3)[:, None], n_vert, n_edge)).astype(f32)
    n = n / (np.linalg.norm(n, axis=-1, keepdims=True).astype(f32) + EPS)
    return np.sum((g_verts - npt) * n, axis=1).astype(f32)


def kernel(batch_garment_verts, batch_body_verts, body_faces, _profile=None):
    from concourse.bass_utils import run_bass_kernel_spmd

    batch_garment_verts = np.asarray(batch_garment_verts, dtype=np.float32)
    batch_body_verts = np.asarray(batch_body_verts, dtype=np.float32)
    body_faces = np.asarray(body_faces)

    nc = _get_nc()
    in_maps = _core_inputs(batch_garment_verts, batch_body_verts, body_faces)
    kwargs = dict(_profile) if _profile else {}
    res = run_bass_kernel_spmd(nc, in_maps, list(range(NCORES)), **kwargs)
    if _profile is not None:
        _CACHE["last_results"] = res

    vals = np.stack([r["out_val"] for r in res.results])  # [8,B,FT,128,G]
    # noisy device score for local face ft*128+p of core c
    flat = vals.transpose(1, 4, 0, 2, 3).reshape(B, G, NCORES * FPAD)
    local = np.arange(NCORES * FPAD) % FPAD
    flat = np.where(local[None, None, :] < FC, flat, np.inf)
    out = np.empty((B, G), np.float32)
    for b in range(B):
        top = np.argpartition(flat[b], TOPM, axis=1)[:, :TOPM]  # [G, M]
        cand = (top // FPAD) * FC + (top % FPAD)                # global face id
        dref = _d2_exact64_cand(batch_garment_verts[b], batch_body_verts[b],
                                body_faces, cand)
        mn = dref.min(axis=1, keepdims=True)
        sel = np.where(dref == mn, cand, F + 1)
        tri = sel.min(axis=1)
        out[b] = _host_finish(batch_garment_verts[b], batch_body_verts[b],
                              body_faces, tri)
    return out


# revision 34
# speedup vs baseline: 1.0310x; 1.0310x over previous
"""Accurate SDF (garment-to-body signed distance) on 8 Trainium2 cores — v2.

Faces sharded 8 ways (1722/core, padded to 14*128); every core scores all
B*G garment points against its faces and returns per-PSUM-partition running
minima [B, 128, G] (no on-device argmin). Host takes the top-M partitions
per point by device score, exactly re-ranks their 14 faces each in fp64,
and finishes (region code, normals, sign) with the reference formulas.

Device math per (face f, point g), with faces on partitions and g on the
free dim (moving rows P5 = [px, py, pz, 1, |p|^2]):
  edge e (seg anchor v_e, unit dir u_e, length L_e):
    U_e = u_e.(p - v_e)                (fp32 matmul)
    T_e = clamp(U_e, 0, L_e)           (relu on Act + min on DVE/Pool)
    w_e = T_e*(2U_e - T_e)             so d2_e = |p - v_e|^2 - w_e
  A    = |p - a|^2                     (fp32 matmul, |p|^2 row)
  A_b  = A + D',  D' = -2 L_ab U_ab + L_ab^2   (Act scale/bias from U_ab)
  face: h = n^.(p - a)  (fp32 matmul), score h^2, masked by the sign of
    vb, vc, va = den - vb - vc (row-normalized fp32r matmuls) via a
    BIG*relu(-min(...)) penalty.
  sc = min(A - max(w_ab, w_ca), A_b - w_bc, h^2 + penalty)
  best[partition] = min over ft tiles  ->  DMA out per (b, gchunk).
"""

import numpy as np

B, G, V, F = 2, 1024, 6890, 13776
NCORES = 8
FC = F // NCORES            # 1722 faces per core
FTILES = 14                 # ceil(1722/128)
FPAD = FTILES * 128         # 1792
GCHUNK = 512
NMM5 = 5                    # fp32 matmuls: U_ab, U_ca, U_bc, A, h
NMM3 = 3                    # fp32r matmuls: vb, vc, va
NCST = 5                    # ptr consts: L_ab, L_ca, L_bc, -2L_ab, L_ab^2
W5COLS = B * FTILES * 2 * 128      # 2 col-blocks, 3+2 sets at part 0/32/64
W3COLS = B * FTILES * 128          # 1 col-block, 3 sets at part 0/32/64
NPART = 69                         # weight/moving tiles span partitions 0..68
CSTCOLS = B * FTILES * NCST
BIG = np.float32(1e6)
INF = np.float32(3e38)
TOPM = 32                   # host: partitions re-ranked exactly per point

_CACHE = {}


def _build_bass():
    import concourse.bass as bass
    import concourse.bacc as bacc
    import concourse.mybir as mybir
    from concourse.tile import TileContext

    dt = mybir.dt.float32
    dtr = mybir.dt.float32r
    dth = mybir.dt.float16
    Alu = mybir.AluOpType
    Act = mybir.ActivationFunctionType

    nc = bacc.Bacc()

    w5h_d = nc.declare_dram_parameter("w5h", [NPART, W5COLS], dtr, isOutput=False)
    w5l_d = nc.declare_dram_parameter("w5l", [NPART, W5COLS], dtr, isOutput=False)
    w3_d = nc.declare_dram_parameter("w3r", [NPART, W3COLS], dtr, isOutput=False)
    p5h_d = nc.declare_dram_parameter("p5h", [NPART, B * G], dtr, isOutput=False)
    p5l_d = nc.declare_dram_parameter("p5l", [NPART, B * G], dtr, isOutput=False)
    cst_d = nc.declare_dram_parameter("cst", [128, CSTCOLS], dt, isOutput=False)
    oval_d = nc.declare_dram_parameter("out_val", [B, FTILES, 128, G], dt,
                                   isOutput=True)

    Vv = nc.vector
    Gg = nc.gpsimd
    Ss = nc.scalar
    Tt = nc.tensor
    Sy = nc.sync

    with TileContext(nc) as tc:
        with (
            tc.tile_pool(name="cpool", bufs=1) as cpool,
            tc.tile_pool(name="work", bufs=1) as work,
            tc.tile_pool(name="mm", bufs=2, space="PSUM") as mm,
        ):
            w5h_s = cpool.tile([NPART, W5COLS], dtr, name="w5h_s")
            w5l_s = cpool.tile([NPART, W5COLS], dtr, name="w5l_s")
            w3_s = cpool.tile([NPART, W3COLS], dtr, name="w3_s")
            p5h_s = cpool.tile([NPART, B * G], dtr, name="p5h_s")
            p5l_s = cpool.tile([NPART, B * G], dtr, name="p5l_s")
            cst_s = cpool.tile([128, CSTCOLS], dt, name="cst_s")
            Sy.dma_start(w5h_s[:], w5h_d[:])
            Sy.dma_start(w5l_s[:], w5l_d[:])
            Sy.dma_start(w3_s[:], w3_d[:])
            Sy.dma_start(p5h_s[:], p5h_d[:])
            Sy.dma_start(p5l_s[:], p5l_d[:])
            Sy.dma_start(cst_s[:], cst_d[:])

            def W5(b, ft, m, lo=False):
                blk, off = divmod(m, 3)
                c = ((b * FTILES + ft) * 2 + blk) * 128
                ws = w5l_s if lo else w5h_s
                return ws[off * 32:off * 32 + NMM5, c:c + 128]

            def W3(b, ft, m):
                c = (b * FTILES + ft) * 128
                return w3_s[m * 32:m * 32 + NMM5, c:c + 128]

            def CST(b, ft, j):
                c = (b * FTILES + ft) * NCST + j
                return cst_s[:, c:c + 1]

            GW = 2 * GCHUNK                     # fused width: both gchunks of b
            sh = [128, GW]

            def emit(b, ft):
                """One face-tile iteration over all G points of batch b.

                Matmuls run as 2x512-wide halves into [128,1024] psum tiles
                (2 banks each); two psum names x bufs=2 = 8 banks, rotated so
                every wait lands on an early Act drain. All elementwise ops
                run fused at width 1024."""
                g0 = b * G

                def MM(name, m, compensated=True):
                    t = mm.tile(sh, dt, name=name)
                    for h in range(2):
                        dst = t[:, h * GCHUNK:(h + 1) * GCHUNK]
                        if compensated:
                            blk, off = divmod(m, 3)
                            ph = p5h_s[off * 32:off * 32 + NMM5,
                                       g0 + h * GCHUNK:g0 + (h + 1) * GCHUNK]
                            pl = p5l_s[off * 32:off * 32 + NMM5,
                                       g0 + h * GCHUNK:g0 + (h + 1) * GCHUNK]
                            Tt.matmul(dst, W5(b, ft, m), ph,
                                      start=True, stop=False)
                            Tt.matmul(dst, W5(b, ft, m), pl,
                                      start=False, stop=False)
                            Tt.matmul(dst, W5(b, ft, m, lo=True), ph,
                                      start=False, stop=True)
                        else:
                            ph = p5h_s[m * 32:m * 32 + NMM5,
                                       g0 + h * GCHUNK:g0 + (h + 1) * GCHUNK]
                            Tt.matmul(dst, W3(b, ft, m), ph,
                                      start=True, stop=True)
                    return t

                def wt(nm, bufs=1, dtype=None):
                    return work.tile(sh, dtype or dt, name=nm, bufs=bufs)

                u_ab = MM("p1", 0)
                u_ca = MM("p2", 1)
                u_bc = MM("p1", 2)
                vcm = MM("p2", 1, compensated=False)
                am = MM("p1", 3)
                hm = MM("p2", 4)
                abm = MM("p1", 5)
                vbm = MM("p2", 0, compensated=False)
                vam = MM("p1", 2, compensated=False)
                # Act: drain psum fast (order matches psum rotation)
                r1ab = wt("r1ab", bufs=2); r1ca = wt("r1ca", bufs=2)
                r1bc = wt("r1bc")
                sf = wt("sf"); ab_s = wt("ab_s")
                a_s = wt("a_s")
                vb_s = wt("vb_s", dtype=dth); vc_s = wt("vc_s", dtype=dth)
                va_s = wt("va_s", dtype=dth)
                Ss.activation(r1ab[:], u_ab[:], Act.Relu)
                Ss.activation(r1ca[:], u_ca[:], Act.Relu)
                Ss.activation(r1bc[:], u_bc[:], Act.Relu)
                Ss.activation(vc_s[:], vcm[:], Act.Identity)
                Ss.activation(a_s[:], am[:], Act.Identity)
                Ss.activation(sf[:], hm[:], Act.Square)
                Ss.activation(ab_s[:], abm[:], Act.Identity)
                Ss.activation(vb_s[:], vbm[:], Act.Identity)
                Ss.activation(va_s[:], vam[:], Act.Identity)
                # clamp T = min(relu(U), L)
                t_ab = wt("t_ab"); t_ca = wt("t_ca"); t_bc = wt("t_bc")
                Vv.tensor_scalar(t_ab[:], r1ab[:], CST(b, ft, 0), None, Alu.min)
                Vv.tensor_scalar(t_ca[:], r1ca[:], CST(b, ft, 1), None, Alu.min)
                Vv.tensor_scalar(t_bc[:], r1bc[:], CST(b, ft, 2), None, Alu.min)
                # z = 2*relu(U) - T  (== 2U - T wherever T != 0)
                z_ab = wt("z_ab"); z_ca = wt("z_ca"); z_bc = wt("z_bc")
                Vv.scalar_tensor_tensor(z_ab[:], r1ab[:], 2.0, t_ab[:],
                                        Alu.mult, Alu.subtract)
                Vv.scalar_tensor_tensor(z_ca[:], r1ca[:], 2.0, t_ca[:],
                                        Alu.mult, Alu.subtract)
                Vv.scalar_tensor_tensor(z_bc[:], r1bc[:], 2.0, t_bc[:],
                                        Alu.mult, Alu.subtract)
                w_ab = wt("w_ab"); w_ca = wt("w_ca")
                w_bc = wt("w_bc")
                Gg.tensor_tensor(w_ab[:], t_ab[:], z_ab[:], Alu.mult)
                Gg.tensor_tensor(w_ca[:], t_ca[:], z_ca[:], Alu.mult)
                Gg.tensor_tensor(w_bc[:], t_bc[:], z_bc[:], Alu.mult)
                # face mask: penalty = BIG*relu(-min(vb,vc,va))
                mn1 = wt("mn1", dtype=dth); mn2 = wt("mn2", dtype=dth)
                rneg = wt("rneg"); sfm = wt("sfm")
                Vv.tensor_tensor(mn1[:], vb_s[:], vc_s[:], Alu.min)
                Vv.tensor_tensor(mn2[:], mn1[:], va_s[:], Alu.min)
                Ss.activation(rneg[:], mn2[:], Act.Relu, scale=-float(BIG))
                # combine
                mx = wt("mx"); e1 = wt("e1"); e2 = wt("e2")
                em = wt("em"); sc = wt("sc", bufs=2)
                Vv.tensor_tensor(sfm[:], rneg[:], sf[:], Alu.add)
                Vv.tensor_tensor(mx[:], w_ab[:], w_ca[:], Alu.max)
                Gg.tensor_tensor(e1[:], a_s[:], mx[:], Alu.subtract)
                Gg.tensor_tensor(e2[:], ab_s[:], w_bc[:], Alu.subtract)
                Vv.tensor_tensor(em[:], e1[:], e2[:], Alu.min)
                Vv.tensor_tensor(sc[:], em[:], sfm[:], Alu.min)
                Sy.dma_start(oval_d[b, ft], sc[:])

            for b in range(B):
                for ft in range(FTILES):
                    emit(b, ft)
    nc.finalize()
    return nc


def _get_nc():
    if "nc" not in _CACHE:
        _CACHE["nc"] = _build_bass()
    return _CACHE["nc"]


def _round_fp32r(x):
    """Round fp32 -> fp32r container (11-bit mantissa, RNE)."""
    u = np.ascontiguousarray(x, np.float32).view(np.uint32)
    base = u & np.uint32(0xFFFFF000)
    low = u & np.uint32(0x00000FFF)
    half = np.uint32(0x800)
    lsb = (base >> np.uint32(12)) & np.uint32(1)
    up = (low > half) | ((low == half) & (lsb == 1))
    return np.where(up, base + np.uint32(0x1000), base).view(np.float32)


def _core_inputs(batch_garment_verts, batch_body_verts, body_faces):
    f32 = np.float32
    gv = batch_garment_verts.astype(f32)
    p5 = np.concatenate(
        [gv.transpose(0, 2, 1),                       # [B,3,G]
         np.ones((B, 1, G), f32),
         np.sum(gv * gv, -1, dtype=f32)[:, None, :]], 1
    ).reshape(B * NMM5, G)
    p5 = np.ascontiguousarray(
        p5.reshape(B, NMM5, G).transpose(1, 0, 2)).reshape(NMM5, B * G)
    p5x = np.zeros((69, B * G), f32)            # replicate at part 0/32/64
    for o in range(3):
        p5x[o * 32:o * 32 + NMM5] = p5
    p5 = p5x
    p5h = _round_fp32r(p5)
    p5l = _round_fp32r(p5 - p5h)

    in_maps = []
    for c in range(NCORES):
        sl = slice(c * FC, (c + 1) * FC)
        w5 = np.zeros((69, W5COLS), f32)
        w3 = np.zeros((69, W3COLS), f32)
        cst = np.zeros((128, CSTCOLS), f32)
        for b in range(B):
            fv = batch_body_verts[b].astype(f32)[body_faces[sl]]  # [FC,3,3]
            a, bb, cc = fv[:, 0], fv[:, 1], fv[:, 2]
            n = np.cross((bb - a).astype(np.float64),
                         (cc - a).astype(np.float64))
            nn = np.linalg.norm(n, axis=1)
            ab, ac = (bb - a).astype(f32), (cc - a).astype(f32)
            naa = np.sum(ab * ab, -1, dtype=f32)
            nab = np.sum(ab * ac, -1, dtype=f32)
            ncc = np.sum(ac * ac, -1, dtype=f32)
            den = (naa * ncc - nab * nab).astype(f32)
            degen = (den < f32(1e-4)) | (nn < 1e-10)
            nh = np.where(degen[:, None], 0,
                          n / np.maximum(nn, 1e-30)[:, None]).astype(f32)

            def pads(rows):  # [FC,5] -> [5, FPAD] padded
                out = np.zeros((NMM5, FPAD), f32)
                out[:, :FC] = rows.T
                return out

            Ls = {}
            Ws = {}
            for nm, (ea, eb) in (("ab", (a, bb)), ("ca", (a, cc)),
                                 ("bc", (bb, cc))):
                ed = (eb - ea).astype(f32)
                L = np.linalg.norm(ed.astype(np.float64), axis=1).astype(f32)
                u = np.where(L[:, None] > 0,
                             ed / np.maximum(L, f32(1e-30))[:, None], 0)
                Ws[nm] = pads(np.concatenate(
                    [u, (-np.sum(u * ea, -1, dtype=f32))[:, None],
                     np.zeros((FC, 1), f32)], 1))
                Lp = np.zeros(FPAD, f32)
                Lp[:FC] = L
                Ls[nm] = Lp
            wa_rows = np.concatenate(
                [-2 * a, np.sum(a * a, -1, dtype=f32)[:, None],
                 np.ones((FC, 1), f32)], 1)
            WA = pads(wa_rows)
            WA[3, FC:] = f32(1e30)                       # pad faces: huge A
            wab_rows = np.concatenate(
                [-2 * bb, np.sum(bb * bb, -1, dtype=f32)[:, None],
                 np.ones((FC, 1), f32)], 1)
            WAb = pads(wab_rows)
            WAb[3, FC:] = f32(1e30)
            Wh = pads(np.concatenate(
                [nh, (-np.sum(nh * a, -1, dtype=f32))[:, None],
                 np.zeros((FC, 1), f32)], 1))
            wvb = (ncc[:, None] * ab - nab[:, None] * ac).astype(f32)
            wvc = (naa[:, None] * ac - nab[:, None] * ab).astype(f32)
            cvb = (-np.sum(wvb * a, -1, dtype=f32))
            cvc = (-np.sum(wvc * a, -1, dtype=f32))
            wva = (-(wvb + wvc)).astype(f32)
            cva = (den - cvb - cvc).astype(f32)
            W3L = []
            for w_, c_ in ((wvb, cvb), (wvc, cvc), (wva, cva)):
                s = np.maximum(np.linalg.norm(w_, axis=1), f32(1e-30)).astype(f32)
                W3L.append(pads(np.concatenate(
                    [w_ / s[:, None], (c_ / s)[:, None],
                     np.zeros((FC, 1), f32)], 1)))
            Wvb, Wvc, Wva = W3L
            Wva[:, :FC][:, degen] = 0.0
            Wva[3, :FC][degen] = -1.0
            Wva[:, FC:] = 0.0
            Wva[3, FC:] = -1.0                           # pad faces: outside

            mm5 = [Ws["ab"], Ws["ca"], Ws["bc"], WA, Wh, WAb]
            mm3 = [Wvb, Wvc, Wva]
            for ft in range(FTILES):
                fsl = slice(ft * 128, (ft + 1) * 128)
                for m in range(6):
                    blk, off = divmod(m, 3)
                    c0 = ((b * FTILES + ft) * 2 + blk) * 128
                    w5[off * 32:off * 32 + NMM5, c0:c0 + 128] = mm5[m][:, fsl]
                for m in range(NMM3):
                    c0 = (b * FTILES + ft) * 128
                    w3[m * 32:m * 32 + NMM5, c0:c0 + 128] = mm3[m][:, fsl]
                c0 = (b * FTILES + ft) * NCST
                cst[:, c0 + 0] = Ls["ab"][fsl]
                cst[:, c0 + 1] = Ls["ca"][fsl]
                cst[:, c0 + 2] = Ls["bc"][fsl]
                cst[:, c0 + 3] = -2 * Ls["ab"][fsl]
                cst[:, c0 + 4] = Ls["ab"][fsl] ** 2
        w5h = _round_fp32r(w5)
        w5l = _round_fp32r(w5 - w5h)
        in_maps.append({"w5h": w5h, "w5l": w5l, "w3r": _round_fp32r(w3),
                        "p5h": p5h, "p5l": p5l, "cst": cst})
    return in_maps


def _d2_exact64_cand(p, bverts, faces, cand):
    """Exact fp64 point-triangle dist^2 for candidate faces. cand [G,C]."""
    fv = bverts[faces[cand]].astype(np.float64)      # [G,C,3,3]
    a, b, c = fv[:, :, 0], fv[:, :, 1], fv[:, :, 2]
    q = p.astype(np.float64)[:, None, :]
    best = np.full(cand.shape, np.inf)
    for ea, eb in ((a, b), (b, c), (c, a)):
        ed = eb - ea
        L2 = np.sum(ed * ed, -1)
        pe = q - ea
        t = np.clip(np.sum(pe * ed, -1) / np.maximum(L2, 1e-300), 0, 1)
        d = pe - t[..., None] * ed
        best = np.minimum(best, np.sum(d * d, -1))
    ab, ac = b - a, c - a
    n = np.cross(ab, ac)
    naa = np.sum(ab * ab, -1); nab = np.sum(ab * ac, -1)
    ncc = np.sum(ac * ac, -1)
    den = naa * ncc - nab * nab
    pa = q - a
    d1 = np.sum(pa * ab, -1); d2_ = np.sum(pa * ac, -1)
    vb = ncc * d1 - nab * d2_; vc = naa * d2_ - nab * d1
    va = den - vb - vc
    inside = (vb >= 0) & (vc >= 0) & (va >= 0) & (den > 1e-300)
    hn = np.sum(pa * n, -1)
    h2 = hn * hn / np.maximum(den, 1e-300)
    return np.where(inside, np.minimum(best, h2), best)


def _host_finish(g_verts, b_verts, faces, tri):
    """Exact reference finish for the winning face of each garment point."""
    f32 = np.float32
    EPS = f32(1e-10)

    def safe(x):
        return np.where(np.abs(x) < 1e-12, f32(1e-12), x).astype(f32)

    fverts = b_verts[faces]
    a_, b_, c_ = fverts[:, 0], fverts[:, 1], fverts[:, 2]
    fn_raw = np.cross(b_ - a_, c_ - a_).astype(f32)
    vn = np.zeros_like(b_verts)
    for k in range(3):
        np.add.at(vn, faces[:, k], fn_raw)
    vn = vn / (np.linalg.norm(vn, axis=-1, keepdims=True).astype(f32) + EPS)
    fn = fn_raw / (np.linalg.norm(fn_raw, axis=-1, keepdims=True).astype(f32) + EPS)

    a = a_[tri]; bb = b_[tri]; cc = c_[tri]
    q = g_verts
    ab = bb - a; ac = cc - a
    ap = q - a
    d1 = np.sum(ab * ap, -1); d2 = np.sum(ac * ap, -1)
    bp = q - bb
    d3 = np.sum(ab * bp, -1); d4 = np.sum(ac * bp, -1)
    cp = q - cc
    d5 = np.sum(ab * cp, -1); d6 = np.sum(ac * cp, -1)
    vc = d1 * d4 - d3 * d2
    vb = d5 * d2 - d1 * d6
    va = d3 * d6 - d5 * d4
    denom = safe(va + vb + vc)
    v, w = (vb / denom).astype(f32), (vc / denom).astype(f32)
    part = np.zeros(v.shape, np.int32)
    t_bc = ((d4 - d3) / safe((d4 - d3) + (d5 - d6))).astype(f32)
    m = (va <= 0) & (d4 - d3 >= 0) & (d5 - d6 >= 0)
    v = np.where(m, 1.0 - t_bc, v).astype(f32)
    w = np.where(m, t_bc, w).astype(f32)
    part = np.where(m, 2, part)
    t_ac = (d2 / safe(d2 - d6)).astype(f32)
    m = (vb <= 0) & (d2 >= 0) & (d6 <= 0)
    v = np.where(m, 0.0, v).astype(f32)
    w = np.where(m, t_ac, w).astype(f32)
    part = np.where(m, 3, part)
    m = (d6 >= 0) & (d5 <= d6)
    v = np.where(m, 0.0, v).astype(f32)
    w = np.where(m, 1.0, w).astype(f32)
    part = np.where(m, 6, part)
    t_ab = (d1 / safe(d1 - d3)).astype(f32)
    m = (vc <= 0) & (d1 >= 0) & (d3 <= 0)
    v = np.where(m, t_ab, v).astype(f32)
    w = np.where(m, 0.0, w).astype(f32)
    part = np.where(m, 1, part)
    m = (d3 >= 0) & (d4 <= d3)
    v = np.where(m, 1.0, v).astype(f32)
    w = np.where(m, 0.0, w).astype(f32)
    part = np.where(m, 5, part)
    m = (d1 <= 0) & (d2 <= 0)
    v = np.where(m, 0.0, v).astype(f32)
    w = np.where(m, 0.0, w).astype(f32)
    part = np.where(m, 4, part)
    npt = a + v[:, None] * ab + w[:, None] * ac

    fidx = faces[tri]
    gar = np.arange(len(tri))
    take = lambda col: vn[fidx[gar, col]]
    n_face = fn[tri]
    n_vert = take(np.clip(part - 4, 0, 2))
    n_edge = take(np.clip(part - 1, 0, 2)) + take(np.mod(part, 3))
    n = np.where((part == 0)[:, None], n_face,
                 np.where((part > 3)[:, None], n_vert, n_edge)).astype(f32)
    n = n / (np.linalg.norm(n, axis=-1, keepdims=True).astype(f32) + EPS)
    return np.sum((g_verts - npt) * n, axis=1).astype(f32)


def kernel(batch_garment_verts, batch_body_verts, body_faces, _profile=None):
    from concourse.bass_utils import run_bass_kernel_spmd

    batch_garment_verts = np.asarray(batch_garment_verts, dtype=np.float32)
    batch_body_verts = np.asarray(batch_body_verts, dtype=np.float32)
    body_faces = np.asarray(body_faces)

    nc = _get_nc()
    in_maps = _core_inputs(batch_garment_verts, batch_body_verts, body_faces)
    kwargs = dict(_profile) if _profile else {}
    res = run_bass_kernel_spmd(nc, in_maps, list(range(NCORES)), **kwargs)
    if _profile is not None:
        _CACHE["last_results"] = res

    vals = np.stack([r["out_val"] for r in res.results])  # [8,B,FT,128,G]
    # noisy device score for local face ft*128+p of core c
    flat = vals.transpose(1, 4, 0, 2, 3).reshape(B, G, NCORES * FPAD)
    local = np.arange(NCORES * FPAD) % FPAD
    flat = np.where(local[None, None, :] < FC, flat, np.inf)
    out = np.empty((B, G), np.float32)
    for b in range(B):
        top = np.argpartition(flat[b], TOPM, axis=1)[:, :TOPM]  # [G, M]
        cand = (top // FPAD) * FC + (top % FPAD)                # global face id
        dref = _d2_exact64_cand(batch_garment_verts[b], batch_body_verts[b],
                                body_faces, cand)
        mn = dref.min(axis=1, keepdims=True)
        sel = np.where(dref == mn, cand, F + 1)
        tri = sel.min(axis=1)
        out[b] = _host_finish(batch_garment_verts[b], batch_body_verts[b],
                              body_faces, tri)
    return out


# revision 35
# speedup vs baseline: 1.1704x; 1.1351x over previous
"""Accurate SDF (garment-to-body signed distance) on 8 Trainium2 cores — v2.

Faces sharded 8 ways (1722/core, padded to 14*128); every core scores all
B*G garment points against its faces and returns per-PSUM-partition running
minima [B, 128, G] (no on-device argmin). Host takes the top-M partitions
per point by device score, exactly re-ranks their 14 faces each in fp64,
and finishes (region code, normals, sign) with the reference formulas.

Device math per (face f, point g), with faces on partitions and g on the
free dim (moving rows P5 = [px, py, pz, 1, |p|^2]):
  edge e (seg anchor v_e, unit dir u_e, length L_e):
    U_e = u_e.(p - v_e)                (fp32 matmul)
    T_e = clamp(U_e, 0, L_e)           (relu on Act + min on DVE/Pool)
    w_e = T_e*(2U_e - T_e)             so d2_e = |p - v_e|^2 - w_e
  A    = |p - a|^2                     (fp32 matmul, |p|^2 row)
  A_b  = A + D',  D' = -2 L_ab U_ab + L_ab^2   (Act scale/bias from U_ab)
  face: h = n^.(p - a)  (fp32 matmul), score h^2, masked by the sign of
    vb, vc, va = den - vb - vc (row-normalized fp32r matmuls) via a
    BIG*relu(-min(...)) penalty.
  sc = min(A - max(w_ab, w_ca), A_b - w_bc, h^2 + penalty)
  best[partition] = min over ft tiles  ->  DMA out per (b, gchunk).
"""

import numpy as np

B, G, V, F = 2, 1024, 6890, 13776
NCORES = 8
FC = F // NCORES            # 1722 faces per core
FTILES = 14                 # ceil(1722/128)
FPAD = FTILES * 128         # 1792
GCHUNK = 512
NMM5 = 5                    # fp32 matmuls: U_ab, U_ca, U_bc, A, h
NMM3 = 3                    # fp32r matmuls: vb, vc, va
NCST = 5                    # ptr consts: L_ab, L_ca, L_bc, -2L_ab, L_ab^2
W5COLS = B * FTILES * 2 * 128      # 2 col-blocks, 3+2 sets at part 0/32/64
W3COLS = B * FTILES * 128          # 1 col-block, 3 sets at part 0/32/64
NPART = 69                         # weight/moving tiles span partitions 0..68
CSTCOLS = B * FTILES * NCST
BIG = np.float32(1e6)
INF = np.float32(3e38)
TOPM = 32                   # host: partitions re-ranked exactly per point

_CACHE = {}


def _build_bass():
    import concourse.bass as bass
    import concourse.bacc as bacc
    import concourse.mybir as mybir
    from concourse.tile import TileContext

    dt = mybir.dt.float32
    dtr = mybir.dt.float32r
    dth = mybir.dt.float16
    Alu = mybir.AluOpType
    Act = mybir.ActivationFunctionType

    nc = bacc.Bacc()

    w5h_d = nc.declare_dram_parameter("w5h", [NPART, W5COLS], dtr, isOutput=False)
    w5l_d = nc.declare_dram_parameter("w5l", [NPART, W5COLS], dtr, isOutput=False)
    w3_d = nc.declare_dram_parameter("w3r", [NPART, W3COLS], dtr, isOutput=False)
    p5h_d = nc.declare_dram_parameter("p5h", [NPART, B * G], dtr, isOutput=False)
    p5l_d = nc.declare_dram_parameter("p5l", [NPART, B * G], dtr, isOutput=False)
    cst_d = nc.declare_dram_parameter("cst", [128, CSTCOLS], dt, isOutput=False)
    oval_d = nc.declare_dram_parameter("out_val", [B, FTILES, 128, G], dt,
                                   isOutput=True)

    Vv = nc.vector
    Gg = nc.gpsimd
    Ss = nc.scalar
    Tt = nc.tensor
    Sy = nc.sync

    with TileContext(nc) as tc:
        with (
            tc.tile_pool(name="cpool", bufs=1) as cpool,
            tc.tile_pool(name="work", bufs=1) as work,
            tc.tile_pool(name="mm", bufs=2, space="PSUM") as mm,
        ):
            w5h_s = cpool.tile([NPART, W5COLS], dtr, name="w5h_s")
            w5l_s = cpool.tile([NPART, W5COLS], dtr, name="w5l_s")
            w3_s = cpool.tile([NPART, W3COLS], dtr, name="w3_s")
            p5h_s = cpool.tile([NPART, B * G], dtr, name="p5h_s")
            p5l_s = cpool.tile([NPART, B * G], dtr, name="p5l_s")
            cst_s = cpool.tile([128, CSTCOLS], dt, name="cst_s")
            Sy.dma_start(w5h_s[:], w5h_d[:])
            Sy.dma_start(w5l_s[:], w5l_d[:])
            Sy.dma_start(w3_s[:], w3_d[:])
            Sy.dma_start(p5h_s[:], p5h_d[:])
            Sy.dma_start(p5l_s[:], p5l_d[:])
            Sy.dma_start(cst_s[:], cst_d[:])

            def W5(b, ft, m, lo=False):
                blk, off = divmod(m, 3)
                c = ((b * FTILES + ft) * 2 + blk) * 128
                ws = w5l_s if lo else w5h_s
                return ws[off * 32:off * 32 + NMM5, c:c + 128]

            def W3(b, ft, m):
                c = (b * FTILES + ft) * 128
                return w3_s[m * 32:m * 32 + NMM5, c:c + 128]

            def CST(b, ft, j):
                c = (b * FTILES + ft) * NCST + j
                return cst_s[:, c:c + 1]

            GW = 2 * GCHUNK                     # fused width: both gchunks of b
            sh = [128, GW]

            def emit(b, ft):
                """One face-tile iteration over all G points of batch b.

                Matmuls run as 2x512-wide halves into [128,1024] psum tiles
                (2 banks each); two psum names x bufs=2 = 8 banks, rotated so
                every wait lands on an early Act drain. All elementwise ops
                run fused at width 1024."""
                g0 = b * G

                def MM(name, m, compensated=True):
                    t = mm.tile(sh, dt, name=name)
                    for h in range(2):
                        dst = t[:, h * GCHUNK:(h + 1) * GCHUNK]
                        if compensated:
                            blk, off = divmod(m, 3)
                            ph = p5h_s[off * 32:off * 32 + NMM5,
                                       g0 + h * GCHUNK:g0 + (h + 1) * GCHUNK]
                            pl = p5l_s[off * 32:off * 32 + NMM5,
                                       g0 + h * GCHUNK:g0 + (h + 1) * GCHUNK]
                            Tt.matmul(dst, W5(b, ft, m), ph,
                                      start=True, stop=False)
                            Tt.matmul(dst, W5(b, ft, m), pl,
                                      start=False, stop=False)
                            Tt.matmul(dst, W5(b, ft, m, lo=True), ph,
                                      start=False, stop=True)
                        else:
                            ph = p5h_s[m * 32:m * 32 + NMM5,
                                       g0 + h * GCHUNK:g0 + (h + 1) * GCHUNK]
                            Tt.matmul(dst, W3(b, ft, m), ph,
                                      start=True, stop=True)
                    return t

                def wt(nm, bufs=1, dtype=None):
                    return work.tile(sh, dtype or dt, name=nm, bufs=bufs)

                u_ab = MM("p1", 0)
                u_ca = MM("p2", 1)
                u_bc = MM("p1", 2)
                vcm = MM("p2", 1, compensated=False)
                am = MM("p1", 3)
                hm = MM("p2", 4)
                abm = MM("p1", 5)
                vbm = MM("p2", 0, compensated=False)
                vam = MM("p1", 2, compensated=False)
                # Act: drain psum fast (order matches psum rotation)
                r1ab = wt("r1ab", bufs=2); r1ca = wt("r1ca", bufs=2)
                r1bc = wt("r1bc")
                sf = wt("sf"); ab_s = wt("ab_s")
                a_s = wt("a_s")
                vb_s = wt("vb_s", dtype=dth); vc_s = wt("vc_s", dtype=dth)
                va_s = wt("va_s", dtype=dth)
                Ss.activation(r1ab[:], u_ab[:], Act.Relu)
                Ss.activation(r1ca[:], u_ca[:], Act.Relu)
                Ss.activation(r1bc[:], u_bc[:], Act.Relu)
                Ss.activation(vc_s[:], vcm[:], Act.Identity)
                Ss.activation(a_s[:], am[:], Act.Identity)
                Ss.activation(sf[:], hm[:], Act.Square)
                Ss.activation(ab_s[:], abm[:], Act.Identity)
                Ss.activation(vb_s[:], vbm[:], Act.Identity)
                Ss.activation(va_s[:], vam[:], Act.Identity)
                # clamp T = min(relu(U), L)
                t_ab = wt("t_ab"); t_ca = wt("t_ca"); t_bc = wt("t_bc")
                Vv.tensor_scalar(t_ab[:], r1ab[:], CST(b, ft, 0), None, Alu.min)
                Vv.tensor_scalar(t_ca[:], r1ca[:], CST(b, ft, 1), None, Alu.min)
                Vv.tensor_scalar(t_bc[:], r1bc[:], CST(b, ft, 2), None, Alu.min)
                # z = 2*relu(U) - T  (== 2U - T wherever T != 0)
                z_ab = wt("z_ab"); z_ca = wt("z_ca"); z_bc = wt("z_bc")
                Vv.scalar_tensor_tensor(z_ab[:], r1ab[:], 2.0, t_ab[:],
                                        Alu.mult, Alu.subtract)
                Vv.scalar_tensor_tensor(z_ca[:], r1ca[:], 2.0, t_ca[:],
                                        Alu.mult, Alu.subtract)
                Vv.scalar_tensor_tensor(z_bc[:], r1bc[:], 2.0, t_bc[:],
                                        Alu.mult, Alu.subtract)
                # face mask: penalty = BIG*relu(-min(vb,vc,va))
                mn1 = wt("mn1", dtype=dth); mn2 = wt("mn2", dtype=dth)
                rneg = wt("rneg")
                Vv.tensor_tensor(mn1[:], vb_s[:], vc_s[:], Alu.min)
                Vv.tensor_tensor(mn2[:], mn1[:], va_s[:], Alu.min)
                Ss.activation(rneg[:], mn2[:], Act.Relu, scale=-float(BIG))
                return dict(b=b, ft=ft, t_ab=t_ab, t_ca=t_ca, t_bc=t_bc,
                            z_ab=z_ab, z_ca=z_ca, z_bc=z_bc, rneg=rneg,
                            sf=sf, a_s=a_s, ab_s=ab_s)

            def emit_s2(c):
                """Stage 2: w products, score combine, DMA out."""
                def wt(nm, bufs=1, dtype=None):
                    return work.tile(sh, dtype or dt, name=nm, bufs=bufs)

                w_ab = wt("w_ab"); w_ca = wt("w_ca")
                w_bc = wt("w_bc")
                Gg.tensor_tensor(w_ab[:], c["t_ab"][:], c["z_ab"][:], Alu.mult)
                Gg.tensor_tensor(w_ca[:], c["t_ca"][:], c["z_ca"][:], Alu.mult)
                Gg.tensor_tensor(w_bc[:], c["t_bc"][:], c["z_bc"][:], Alu.mult)
                mx = wt("mx"); e1 = wt("e1"); e2 = wt("e2")
                em = wt("em"); sfm = wt("sfm"); sc = wt("sc", bufs=2)
                Vv.tensor_tensor(sfm[:], c["rneg"][:], c["sf"][:], Alu.add)
                Vv.tensor_tensor(mx[:], w_ab[:], w_ca[:], Alu.max)
                Gg.tensor_tensor(e1[:], c["a_s"][:], mx[:], Alu.subtract)
                Gg.tensor_tensor(e2[:], c["ab_s"][:], w_bc[:], Alu.subtract)
                Vv.tensor_tensor(em[:], e1[:], e2[:], Alu.min)
                Vv.tensor_tensor(sc[:], em[:], sfm[:], Alu.min)
                Sy.dma_start(oval_d[c["b"], c["ft"]], sc[:])

            steps = [(b, ft) for b in range(B) for ft in range(FTILES)]
            prev = None
            for b, ft in steps:
                ctx = emit(b, ft)
                if prev is not None:
                    emit_s2(prev)
                prev = ctx
            emit_s2(prev)
    nc.finalize()
    return nc


def _get_nc():
    if "nc" not in _CACHE:
        _CACHE["nc"] = _build_bass()
    return _CACHE["nc"]


def _round_fp32r(x):
    """Round fp32 -> fp32r container (11-bit mantissa, RNE)."""
    u = np.ascontiguousarray(x, np.float32).view(np.uint32)
    base = u & np.uint32(0xFFFFF000)
    low = u & np.uint32(0x00000FFF)
    half = np.uint32(0x800)
    lsb = (base >> np.uint32(12)) & np.uint32(1)
    up = (low > half) | ((low == half) & (lsb == 1))
    return np.where(up, base + np.uint32(0x1000), base).view(np.float32)


def _core_inputs(batch_garment_verts, batch_body_verts, body_faces):
    f32 = np.float32
    gv = batch_garment_verts.astype(f32)
    p5 = np.concatenate(
        [gv.transpose(0, 2, 1),                       # [B,3,G]
         np.ones((B, 1, G), f32),
         np.sum(gv * gv, -1, dtype=f32)[:, None, :]], 1
    ).reshape(B * NMM5, G)
    p5 = np.ascontiguousarray(
        p5.reshape(B, NMM5, G).transpose(1, 0, 2)).reshape(NMM5, B * G)
    p5x = np.zeros((69, B * G), f32)            # replicate at part 0/32/64
    for o in range(3):
        p5x[o * 32:o * 32 + NMM5] = p5
    p5 = p5x
    p5h = _round_fp32r(p5)
    p5l = _round_fp32r(p5 - p5h)

    in_maps = []
    for c in range(NCORES):
        sl = slice(c * FC, (c + 1) * FC)
        w5 = np.zeros((69, W5COLS), f32)
        w3 = np.zeros((69, W3COLS), f32)
        cst = np.zeros((128, CSTCOLS), f32)
        for b in range(B):
            fv = batch_body_verts[b].astype(f32)[body_faces[sl]]  # [FC,3,3]
            a, bb, cc = fv[:, 0], fv[:, 1], fv[:, 2]
            n = np.cross((bb - a).astype(np.float64),
                         (cc - a).astype(np.float64))
            nn = np.linalg.norm(n, axis=1)
            ab, ac = (bb - a).astype(f32), (cc - a).astype(f32)
            naa = np.sum(ab * ab, -1, dtype=f32)
            nab = np.sum(ab * ac, -1, dtype=f32)
            ncc = np.sum(ac * ac, -1, dtype=f32)
            den = (naa * ncc - nab * nab).astype(f32)
            degen = (den < f32(1e-4)) | (nn < 1e-10)
            nh = np.where(degen[:, None], 0,
                          n / np.maximum(nn, 1e-30)[:, None]).astype(f32)

            def pads(rows):  # [FC,5] -> [5, FPAD] padded
                out = np.zeros((NMM5, FPAD), f32)
                out[:, :FC] = rows.T
                return out

            Ls = {}
            Ws = {}
            for nm, (ea, eb) in (("ab", (a, bb)), ("ca", (a, cc)),
                                 ("bc", (bb, cc))):
                ed = (eb - ea).astype(f32)
                L = np.linalg.norm(ed.astype(np.float64), axis=1).astype(f32)
                u = np.where(L[:, None] > 0,
                             ed / np.maximum(L, f32(1e-30))[:, None], 0)
                Ws[nm] = pads(np.concatenate(
                    [u, (-np.sum(u * ea, -1, dtype=f32))[:, None],
                     np.zeros((FC, 1), f32)], 1))
                Lp = np.zeros(FPAD, f32)
                Lp[:FC] = L
                Ls[nm] = Lp
            wa_rows = np.concatenate(
                [-2 * a, np.sum(a * a, -1, dtype=f32)[:, None],
                 np.ones((FC, 1), f32)], 1)
            WA = pads(wa_rows)
            WA[3, FC:] = f32(1e30)                       # pad faces: huge A
            wab_rows = np.concatenate(
                [-2 * bb, np.sum(bb * bb, -1, dtype=f32)[:, None],
                 np.ones((FC, 1), f32)], 1)
            WAb = pads(wab_rows)
            WAb[3, FC:] = f32(1e30)
            Wh = pads(np.concatenate(
                [nh, (-np.sum(nh * a, -1, dtype=f32))[:, None],
                 np.zeros((FC, 1), f32)], 1))
            wvb = (ncc[:, None] * ab - nab[:, None] * ac).astype(f32)
            wvc = (naa[:, None] * ac - nab[:, None] * ab).astype(f32)
            cvb = (-np.sum(wvb * a, -1, dtype=f32))
            cvc = (-np.sum(wvc * a, -1, dtype=f32))
            wva = (-(wvb + wvc)).astype(f32)
            cva = (den - cvb - cvc).astype(f32)
            W3L = []
            for w_, c_ in ((wvb, cvb), (wvc, cvc), (wva, cva)):
                s = np.maximum(np.linalg.norm(w_, axis=1), f32(1e-30)).astype(f32)
                W3L.append(pads(np.concatenate(
                    [w_ / s[:, None], (c_ / s)[:, None],
                     np.zeros((FC, 1), f32)], 1)))
            Wvb, Wvc, Wva = W3L
            Wva[:, :FC][:, degen] = 0.0
            Wva[3, :FC][degen] = -1.0
            Wva[:, FC:] = 0.0
            Wva[3, FC:] = -1.0                           # pad faces: outside

            mm5 = [Ws["ab"], Ws["ca"], Ws["bc"], WA, Wh, WAb]
            mm3 = [Wvb, Wvc, Wva]
            for ft in range(FTILES):
                fsl = slice(ft * 128, (ft + 1) * 128)
                for m in range(6):
                    blk, off = divmod(m, 3)
                    c0 = ((b * FTILES + ft) * 2 + blk) * 128
                    w5[off * 32:off * 32 + NMM5, c0:c0 + 128] = mm5[m][:, fsl]
                for m in range(NMM3):
                    c0 = (b * FTILES + ft) * 128
                    w3[m * 32:m * 32 + NMM5, c0:c0 + 128] = mm3[m][:, fsl]
                c0 = (b * FTILES + ft) * NCST
                cst[:, c0 + 0] = Ls["ab"][fsl]
                cst[:, c0 + 1] = Ls["ca"][fsl]
                cst[:, c0 + 2] = Ls["bc"][fsl]
                cst[:, c0 + 3] = -2 * Ls["ab"][fsl]
                cst[:, c0 + 4] = Ls["ab"][fsl] ** 2
        w5h = _round_fp32r(w5)
        w5l = _round_fp32r(w5 - w5h)
        in_maps.append({"w5h": w5h, "w5l": w5l, "w3r": _round_fp32r(w3),
                        "p5h": p5h, "p5l": p5l, "cst": cst})
    return in_maps


def _d2_exact64_cand(p, bverts, faces, cand):
    """Exact fp64 point-triangle dist^2 for candidate faces. cand [G,C]."""
    fv = bverts[faces[cand]].astype(np.float64)      # [G,C,3,3]
    a, b, c = fv[:, :, 0], fv[:, :, 1], fv[:, :, 2]
    q = p.astype(np.float64)[:, None, :]
    best = np.full(cand.shape, np.inf)
    for ea, eb in ((a, b), (b, c), (c, a)):
        ed = eb - ea
        L2 = np.sum(ed * ed, -1)
        pe = q - ea
        t = np.clip(np.sum(pe * ed, -1) / np.maximum(L2, 1e-300), 0, 1)
        d = pe - t[..., None] * ed
        best = np.minimum(best, np.sum(d * d, -1))
    ab, ac = b - a, c - a
    n = np.cross(ab, ac)
    naa = np.sum(ab * ab, -1); nab = np.sum(ab * ac, -1)
    ncc = np.sum(ac * ac, -1)
    den = naa * ncc - nab * nab
    pa = q - a
    d1 = np.sum(pa * ab, -1); d2_ = np.sum(pa * ac, -1)
    vb = ncc * d1 - nab * d2_; vc = naa * d2_ - nab * d1
    va = den - vb - vc
    inside = (vb >= 0) & (vc >= 0) & (va >= 0) & (den > 1e-300)
    hn = np.sum(pa * n, -1)
    h2 = hn * hn / np.maximum(den, 1e-300)
    return np.where(inside, np.minimum(best, h2), best)


def _host_finish(g_verts, b_verts, faces, tri):
    """Exact reference finish for the winning face of each garment point."""
    f32 = np.float32
    EPS = f32(1e-10)

    def safe(x):
        return np.where(np.abs(x) < 1e-12, f32(1e-12), x).astype(f32)

    fverts = b_verts[faces]
    a_, b_, c_ = fverts[:, 0], fverts[:, 1], fverts[:, 2]
    fn_raw = np.cross(b_ - a_, c_ - a_).astype(f32)
    vn = np.zeros_like(b_verts)
    for k in range(3):
        np.add.at(vn, faces[:, k], fn_raw)
    vn = vn / (np.linalg.norm(vn, axis=-1, keepdims=True).astype(f32) + EPS)
    fn = fn_raw / (np.linalg.norm(fn_raw, axis=-1, keepdims=True).astype(f32) + EPS)

    a = a_[tri]; bb = b_[tri]; cc = c_[tri]
    q = g_verts
    ab = bb - a; ac = cc - a
    ap = q - a
    d1 = np.sum(ab * ap, -1); d2 = np.sum(ac * ap, -1)
    bp = q - bb
    d3 = np.sum(ab * bp, -1); d4 = np.sum(ac * bp, -1)
    cp = q - cc
    d5 = np.sum(ab * cp, -1); d6 = np.sum(ac * cp, -1)
    vc = d1 * d4 - d3 * d2
    vb = d5 * d2 - d1 * d6
    va = d3 * d6 - d5 * d4
    denom = safe(va + vb + vc)
    v, w = (vb / denom).astype(f32), (vc / denom).astype(f32)
    part = np.zeros(v.shape, np.int32)
    t_bc = ((d4 - d3) / safe((d4 - d3) + (d5 - d6))).astype(f32)
    m = (va <= 0) & (d4 - d3 >= 0) & (d5 - d6 >= 0)
    v = np.where(m, 1.0 - t_bc, v).astype(f32)
    w = np.where(m, t_bc, w).astype(f32)
    part = np.where(m, 2, part)
    t_ac = (d2 / safe(d2 - d6)).astype(f32)
    m = (vb <= 0) & (d2 >= 0) & (d6 <= 0)
    v = np.where(m, 0.0, v).astype(f32)
    w = np.where(m, t_ac, w).astype(f32)
    part = np.where(m, 3, part)
    m = (d6 >= 0) & (d5 <= d6)
    v = np.where(m, 0.0, v).astype(f32)
    w = np.where(m, 1.0, w).astype(f32)
    part = np.where(m, 6, part)
    t_ab = (d1 / safe(d1 - d3)).astype(f32)
    m = (vc <= 0) & (d1 >= 0) & (d3 <= 0)
    v = np.where(m, t_ab, v).astype(f32)
    w = np.where(m, 0.0, w).astype(f32)
    part = np.where(m, 1, part)
    m = (d3 >= 0) & (d4 <= d3)
    v = np.where(m, 1.0, v).astype(f32)
    w = np.where(m, 0.0, w).astype(f32)
    part = np.where(m, 5, part)
    m = (d1 <= 0) & (d2 <= 0)
    v = np.where(m, 0.0, v).astype(f32)
    w = np.where(m, 0.0, w).astype(f32)
    part = np.where(m, 4, part)
    npt = a + v[:, None] * ab + w[:, None] * ac

    fidx = faces[tri]
    gar = np.arange(len(tri))
    take = lambda col: vn[fidx[gar, col]]
    n_face = fn[tri]
    n_vert = take(np.clip(part - 4, 0, 2))
    n_edge = take(np.clip(part - 1, 0, 2)) + take(np.mod(part, 3))
    n = np.where((part == 0)[:, None], n_face,
                 np.where((part > 3)[:, None], n_vert, n_edge)).astype(f32)
    n = n / (np.linalg.norm(n, axis=-1, keepdims=True).astype(f32) + EPS)
    return np.sum((g_verts - npt) * n, axis=1).astype(f32)


def kernel(batch_garment_verts, batch_body_verts, body_faces, _profile=None):
    from concourse.bass_utils import run_bass_kernel_spmd

    batch_garment_verts = np.asarray(batch_garment_verts, dtype=np.float32)
    batch_body_verts = np.asarray(batch_body_verts, dtype=np.float32)
    body_faces = np.asarray(body_faces)

    nc = _get_nc()
    in_maps = _core_inputs(batch_garment_verts, batch_body_verts, body_faces)
    kwargs = dict(_profile) if _profile else {}
    res = run_bass_kernel_spmd(nc, in_maps, list(range(NCORES)), **kwargs)
    if _profile is not None:
        _CACHE["last_results"] = res

    vals = np.stack([r["out_val"] for r in res.results])  # [8,B,FT,128,G]
    # noisy device score for local face ft*128+p of core c
    flat = vals.transpose(1, 4, 0, 2, 3).reshape(B, G, NCORES * FPAD)
    local = np.arange(NCORES * FPAD) % FPAD
    flat = np.where(local[None, None, :] < FC, flat, np.inf)
    out = np.empty((B, G), np.float32)
    for b in range(B):
        top = np.argpartition(flat[b], TOPM, axis=1)[:, :TOPM]  # [G, M]
        cand = (top // FPAD) * FC + (top % FPAD)                # global face id
        dref = _d2_exact64_cand(batch_garment_verts[b], batch_body_verts[b],
                                body_faces, cand)
        mn = dref.min(axis=1, keepdims=True)
        sel = np.where(dref == mn, cand, F + 1)
        tri = sel.min(axis=1)
        out[b] = _host_finish(batch_garment_verts[b], batch_body_verts[b],
                              body_faces, tri)
    return out
